# revision 1
# baseline (speedup 1.0000x reference)
"""Trainium2 Bass kernel for batched Clifford (Cl(3,1)) geometric product.

out[n, c] = sum_{i,j} CAYLEY[i, j, c] * a[n, i] * b[n, j]

Strategy: Cl(3,1) is isomorphic to M4(R) (real 4x4 matrices). Via a fixed
linear basis change Phi (signed, sparse), the 256-term bilinear blade
product becomes a per-token 4x4 matrix product (64 multiplies). All linear
maps (Phi on both inputs, the k-contraction fused with Phi^-1) run on the
TensorEngine against constant matrices; the only elementwise work is one
[128,512] multiply per 1024 tokens on the VectorEngine.

Data parallel over 8 NeuronCores: each core handles 131072 rows.
"""
import sys

sys.path.insert(0, "/opt/trn_rl_repo")

import numpy as np

N_TOTAL = 1048576
N_CORES = 8
ROWS_PER_CORE = N_TOTAL // N_CORES   # 131072
P = 128
F = 512
NT = ROWS_PER_CORE // 4096           # 32 big tiles of 4096 tokens


# ---------------------------------------------------------------------------
# Constant construction: gamma matrices, Phi iso, expansion/contraction mats
# ---------------------------------------------------------------------------
def _build_consts():
    X = np.array([[0.0, 1.0], [1.0, 0.0]])
    Z = np.array([[1.0, 0.0], [0.0, -1.0]])
    E = np.array([[0.0, 1.0], [-1.0, 0.0]])
    I2 = np.eye(2)
    # generators of Cl(3,1): squares +1,+1,+1,-1, pairwise anticommuting
    g = [np.kron(X, I2), np.kron(Z, I2), np.kron(E, E), np.kron(E, X)]
    M = []
    for I in range(16):
        m = np.eye(4)
        for bit in range(4):
            if (I >> bit) & 1:
                m = m @ g[bit]
        M.append(m)
    Phi = np.stack([m.reshape(16) for m in M], axis=1)   # [(r,c), blade]
    PhiInv = Phi.T / 4.0                                 # orthogonal basis

    Ea = np.zeros((32, 128), np.float32)
    Eb = np.zeros((32, 128), np.float32)
    K4 = np.zeros((128, 32), np.float32)
    for v in range(2):
        for r in range(4):
            for k in range(4):
                for c in range(4):
                    col = v * 64 + r * 16 + k * 4 + c
                    for f in range(16):
                        Ea[v * 16 + f, col] = Phi[r * 4 + k, f]
                        Eb[v * 16 + f, col] = Phi[k * 4 + c, f]
                    for cb in range(16):
                        K4[col, v * 16 + cb] = PhiInv[cb, r * 4 + c]
    Ea4 = np.concatenate([Ea] * 4, axis=0).astype(np.float32)
    Eb4 = np.concatenate([Eb] * 4, axis=0).astype(np.float32)
    return Ea4, Eb4, K4.astype(np.float32), np.eye(128, dtype=np.float32)


def build_program(rows_per_core=ROWS_PER_CORE, repeats=1):
    import concourse.bacc as bacc
    import concourse.mybir as mybir
    from concourse.tile import TileContext

    nt = rows_per_core // 4096
    nc = bacc.Bacc("TRN2", target_bir_lowering=False)
    dt = mybir.dt.float32
    a = nc.dram_tensor("a", [rows_per_core, 16], dt, kind="ExternalInput")
    b = nc.dram_tensor("b", [rows_per_core, 16], dt, kind="ExternalInput")
    cEa = nc.dram_tensor("cEa", [128, 128], dt, kind="ExternalInput")
    cEb = nc.dram_tensor("cEb", [128, 128], dt, kind="ExternalInput")
    cK4 = nc.dram_tensor("cK4", [128, 32], dt, kind="ExternalInput")
    cI = nc.dram_tensor("cI", [128, 128], dt, kind="ExternalInput")
    o = nc.dram_tensor("o", [rows_per_core, 16], dt, kind="ExternalOutput")

    af = a.rearrange("(n g w) c -> n g (w c)", g=P, w=32)
    bf = b.rearrange("(n g w) c -> n g (w c)", g=P, w=32)
    of = o.rearrange("(n g w) c -> n g (w c)", g=P, w=32)

    with TileContext(nc) as tc:
        with tc.tile_pool(name="const", bufs=1) as cpool, \
             tc.tile_pool(name="sb", bufs=3) as sb, \
             tc.tile_pool(name="ps1", bufs=1, space="PSUM") as ps1, \
             tc.tile_pool(name="ps2", bufs=2, space="PSUM") as ps2:
            tEa = cpool.tile([128, 128], dt)
            tEb = cpool.tile([128, 128], dt)
            tK4 = cpool.tile([128, 32], dt)
            tI = cpool.tile([128, 128], dt)
            nc.sync.dma_start(tEa[:, :], cEa[:, :])
            nc.sync.dma_start(tEb[:, :], cEb[:, :])
            nc.sync.dma_start(tK4[:, :], cK4[:, :])
            nc.sync.dma_start(tI[:, :], cI[:, :])

            for _rep in range(repeats):
                for n in range(nt):
                    ta = sb.tile([P, F], dt, tag="ta")
                    tb = sb.tile([P, F], dt, tag="tb")
                    nc.sync.dma_start(ta[:, :], af[n])
                    nc.sync.dma_start(tb[:, :], bf[n])

                    paT = ps1.tile([P, F], dt, tag="paT")
                    pbT = ps1.tile([P, F], dt, tag="pbT")
                    for c in range(4):
                        nc.tensor.transpose(paT[:, 128 * c:128 * (c + 1)],
                                            ta[:, 128 * c:128 * (c + 1)], tI[:, :])
                        nc.tensor.transpose(pbT[:, 128 * c:128 * (c + 1)],
                                            tb[:, 128 * c:128 * (c + 1)], tI[:, :])
                    saT = sb.tile([P, F], dt, tag="saT")
                    sbT = sb.tile([P, F], dt, tag="sbT")
                    nc.scalar.copy(saT[:, :], paT[:, :])
                    nc.scalar.copy(sbT[:, :], pbT[:, :])

                    pout2 = ps1.tile([P, F], dt, tag="pout2")
                    for j in range(4):
                        pA = ps2.tile([P, F], dt, tag="pA")
                        pB = ps2.tile([P, F], dt, tag="pB")
                        nc.tensor.matmul(pA[:, :], tEa[32 * j:32 * (j + 1), :],
                                         saT[32 * j:32 * (j + 1), :],
                                         start=True, stop=True,
                                         tile_position=(32 * j, 0))
                        nc.tensor.matmul(pB[:, :], tEb[32 * j:32 * (j + 1), :],
                                         sbT[32 * j:32 * (j + 1), :],
                                         start=True, stop=True,
                                         tile_position=(32 * j, 0))
                        sA = sb.tile([P, F], dt, tag="sA")
                        nc.scalar.copy(sA[:, :], pA[:, :])
                        spp = sb.tile([P, F], dt, tag="spp")
                        nc.vector.tensor_mul(spp[:, :], sA[:, :], pB[:, :])
                        nc.tensor.matmul(pout2[32 * j:32 * (j + 1), :], tK4[:, :],
                                         spp[:, :], start=True, stop=True,
                                         tile_position=(0, 32 * j))
                    sout2 = sb.tile([P, F], dt, tag="sout2")
                    nc.scalar.copy(sout2[:, :], pout2[:, :])
                    onat = sb.tile([P, F], dt, tag="onat")
                    for c in range(4):
                        poT = ps1.tile([P, 128], dt, tag="poT")
                        nc.tensor.transpose(poT[:, :],
                                            sout2[:, 128 * c:128 * (c + 1)], tI[:, :])
                        nc.scalar.copy(onat[:, 128 * c:128 * (c + 1)], poT[:, :])
                    nc.sync.dma_start(of[n], onat[:, :])

    nc.finalize()
    return nc


_CACHE = {}


def _get_program():
    if "nc" not in _CACHE:
        _CACHE["nc"] = build_program()
        _CACHE["consts"] = _build_consts()
    return _CACHE["nc"], _CACHE["consts"]


def kernel(a, b):
    from concourse.bass_utils import run_bass_kernel_spmd

    a = np.ascontiguousarray(np.asarray(a, dtype=np.float32))
    b = np.ascontiguousarray(np.asarray(b, dtype=np.float32))
    assert a.shape == (N_TOTAL, 16) and b.shape == (N_TOTAL, 16)
    nc, (Ea4, Eb4, K4c, I128) = _get_program()
    in_maps = []
    for i in range(N_CORES):
        sl = slice(i * ROWS_PER_CORE, (i + 1) * ROWS_PER_CORE)
        in_maps.append({"a": a[sl], "b": b[sl], "cEa": Ea4, "cEb": Eb4,
                        "cK4": K4c, "cI": I128})
    res = run_bass_kernel_spmd(nc, in_maps, core_ids=list(range(N_CORES)))
    return np.concatenate([res.results[i]["o"] for i in range(N_CORES)], axis=0)
